# revision 1
# baseline (speedup 1.0000x reference)
"""Trainium2 Bass kernel: cache-distance -> exp kernel -> vocab histogram -> log_softmax.

Math (per cache row i): kern_i = exp(||cache_h[i] - h_t|| / 0.2)
                        cache_p[v] = sum_{i: word_ids[i]==v} kern_i
                        out = log_softmax(cache_p)[None, :]

Device strategy (8 cores, cache sharded along N):
  - cache slice uploaded pre-transposed [D=512, 32768] so D lives on SBUF partitions
  - ACT computes (x - h)^2 via Square activation with per-partition bias -h
  - PE reduces over D with one-hot-column [128,8] lhsT matmuls accumulating
    squared distances for 8 row-groups into one PSUM tile [8, 512]
  - ACT: kern = exp(exp(0.5*ln(25*d2)))  (Ln+Exp share one table set; avoids Sqrt)
  - PE transposes kern into [128, batch] orientation
  - histogram via outer-product matmul: hist[m,n] += sum_p kern_p *
      (m == wid_p % 128) * (n == wid_p // 128), with one-hots built by DVE
      fused tensor_scalar (is_equal, mult) against iota constants; PSUM
      accumulates all 256 batches into a single [128, 394] f32 tile.
Host: sum the 8 partial histograms, reorder to [V], log_softmax (tiny).
"""

import os
import sys

for _p in ("/root/.axon_site", "/root/.axon_site/_ro/trn_rl_repo",
           "/root/.axon_site/_ro/pypackages"):
    if os.path.isdir(_p) and _p not in sys.path:
        sys.path.append(_p)

import numpy as np

VOCAB = 50257
N_CACHE = 262144
D = 512
SMOOTH = 0.2
NCORES = 8
RPC = N_CACHE // NCORES        # 32768 rows per core
NCHUNK = 8
CHUNK = RPC // NCHUNK          # 4096 rows per chunk
GPC = CHUNK // 512             # 8 groups of 512 rows per chunk
NV = 394                       # hist free dim: wid // 128 in [0, 393), padded even
BPC = RPC // 128               # 256 batches of 128 elements per core

_CACHE = {}


def _patch_act_tables():
    """Restrict the activation table-set chooser to
    natural_log_exp_and_others (covers square/ln/exp/copy) so the whole
    kernel needs exactly one ACT_TABLE_LOAD instead of alternating between
    the ln-only and exp-only sets every chunk (~1.3us per reload).
    Set names/order are preserved so act_func_set_id indices stay valid."""
    import concourse.hw_specs as hw_specs
    import concourse.bacc as bacc

    if getattr(hw_specs.get_activation_tables, "_histkernel_patched", False):
        return
    orig = hw_specs.get_activation_tables

    def patched(module_arch):
        tabs = orig(module_arch)
        return {
            name: (fns if name == "natural_log_exp_and_others" else set())
            for name, fns in tabs.items()
        }

    patched._histkernel_patched = True
    hw_specs.get_activation_tables = patched
    bacc.get_activation_tables = patched


def _build_program():
    import concourse.bacc as bacc
    import concourse.tile as tile
    import concourse.mybir as mybir

    _patch_act_tables()

    f32, f16 = mybir.dt.float32, mybir.dt.float16
    AF = mybir.ActivationFunctionType
    ALU = mybir.AluOpType

    nc = bacc.Bacc("TRN2", target_bir_lowering=False, debug=False,
                   num_devices=NCORES)

    xt_d = nc.dram_tensor("xt", [D, RPC], f32, kind="ExternalInput")
    am_d = nc.dram_tensor("am", [128, BPC], f32, kind="ExternalInput")
    bn_d = nc.dram_tensor("bn", [128, BPC], f32, kind="ExternalInput")
    nh_d = nc.dram_tensor("nh", [128, 4], f32, kind="ExternalInput")
    im_d = nc.dram_tensor("im", [128, 128], f16, kind="ExternalInput")
    in_d = nc.dram_tensor("inn", [128, NV], f16, kind="ExternalInput")
    oh_d = nc.dram_tensor("oh", [128, 64], f16, kind="ExternalInput")
    id_d = nc.dram_tensor("idm", [8, 8], f32, kind="ExternalInput")
    hist_d = nc.dram_tensor("hist", [128, NV], f32, kind="ExternalOutput")

    with tile.TileContext(nc) as tc:
        with (
            tc.tile_pool(name="const", bufs=1) as cpool,
            tc.tile_pool(name="x", bufs=6) as xpool,
            tc.tile_pool(name="sq", bufs=4) as sqpool,
            tc.tile_pool(name="s", bufs=6) as spool,
            tc.tile_pool(name="kt", bufs=3) as ktpool,
            tc.tile_pool(name="a", bufs=34) as apool,
            tc.tile_pool(name="b", bufs=34) as bpool,
            tc.tile_pool(name="out", bufs=1) as opool,
            tc.tile_pool(name="pshist", bufs=1, space="PSUM") as pshist,
            tc.tile_pool(name="psdist", bufs=4, space="PSUM") as psdist,
            tc.tile_pool(name="pskt", bufs=2, space="PSUM") as pskt,
        ):
            am = cpool.tile([128, BPC], f32)
            nc.sync.dma_start(am[:], am_d.ap())
            bn = cpool.tile([128, BPC], f32)
            nc.sync.dma_start(bn[:], bn_d.ap())
            nh = cpool.tile([128, 4], f32)
            nc.sync.dma_start(nh[:], nh_d.ap())
            im = cpool.tile([128, 128], f16)
            nc.sync.dma_start(im[:], im_d.ap())
            inn = cpool.tile([128, NV], f16)
            nc.sync.dma_start(inn[:], in_d.ap())
            oh = cpool.tile([128, 64], f16)
            nc.sync.dma_start(oh[:], oh_d.ap())
            idm = cpool.tile([8, 8], f32)
            nc.sync.dma_start(idm[:], id_d.ap())

            hist = pshist.tile([128, NV], f32)
            xt_ap = xt_d.ap()

            # Software-pipelined emission (pairs of chunks). For pair p the
            # kern/transpose/one-hot/hist work is emitted during pair p+1's
            # load+dist phase so the PE stream ([transp][dist][hist]) never
            # waits on the serial ACT chain and HAM stays warm.
            PAIR = 2

            def emit_dist(ch):
                dist = psdist.tile([8, 512], f32)
                mm = 0
                for c in range(4):
                    x = xpool.tile([128, CHUNK], f32)
                    nc.sync.dma_start(
                        x[:],
                        xt_ap[c * 128:(c + 1) * 128,
                              ch * CHUNK:(ch + 1) * CHUNK],
                    )
                    sq = sqpool.tile([128, CHUNK], f16)
                    nc.scalar.activation(sq[:], x[:], AF.Square,
                                         bias=nh[:, c:c + 1])
                    for g in range(GPC):
                        nc.tensor.matmul(
                            dist[:],
                            oh[:, g * 8:(g + 1) * 8],
                            sq[:, g * 512:(g + 1) * 512],
                            start=(mm == 0),
                            stop=(mm == 4 * GPC - 1),
                        )
                        mm += 1
                return dist

            def emit_kern(dist):
                lg = spool.tile([8, 512], f32)
                nc.scalar.activation(lg[:], dist[:], AF.Ln, scale=25.0)
                d5 = spool.tile([8, 512], f32)
                nc.scalar.activation(d5[:], lg[:], AF.Exp, scale=0.5)
                kern = spool.tile([8, 512], f32)
                nc.scalar.activation(kern[:], d5[:], AF.Exp)
                return kern

            def emit_transp(kern):
                ktp = pskt.tile([128, 32], f32)
                for c4 in range(4):
                    nc.tensor.transpose(
                        ktp[:, c4 * 8:(c4 + 1) * 8],
                        kern[:, c4 * 128:(c4 + 1) * 128],
                        idm[:],
                    )
                kt = ktpool.tile([128, 32], f32)
                nc.scalar.copy(kt[:], ktp[:])
                return kt

            def emit_ab(ch, kt):
                abs_ = []
                for bl in range(32):
                    b = ch * 32 + bl
                    A = apool.tile([128, 128], f16)
                    nc.vector.tensor_scalar(
                        A[:], im[:], am[:, b:b + 1], kt[:, bl:bl + 1],
                        ALU.is_equal, ALU.mult,
                    )
                    B = bpool.tile([128, NV], f16)
                    nc.vector.tensor_scalar(
                        B[:], inn[:], bn[:, b:b + 1], None, ALU.is_equal,
                    )
                    abs_.append((b, A, B))
                return abs_

            def emit_hist(abs_):
                for b, A, B in abs_:
                    nc.tensor.matmul(
                        hist[:], A[:], B[:],
                        start=(b == 0), stop=(b == BPC - 1),
                    )

            NPAIR = NCHUNK // PAIR
            prev = None  # [(ch, dist), ...] of previous pair
            for pr in range(NPAIR - 1):
                ab_waves = []
                if prev is not None:
                    kerns = [emit_kern(dist) for _, dist in prev]
                    kts = [emit_transp(k) for k in kerns]
                    ab_waves = [emit_ab(ch, kt)
                                for (ch, _), kt in zip(prev, kts)]
                cur = []
                for ch in range(pr * PAIR, (pr + 1) * PAIR):
                    cur.append((ch, emit_dist(ch)))
                for abs_ in ab_waves:
                    emit_hist(abs_)
                prev = cur
            # Final stretch, chunk-staggered so only the last chunk's
            # histogram work remains after the final DMA completes.
            cA, cB = NCHUNK - 2, NCHUNK - 1
            kerns = [emit_kern(dist) for _, dist in prev]
            kts = [emit_transp(k) for k in kerns]
            ab_prev = [emit_ab(ch, kt) for (ch, _), kt in zip(prev, kts)]
            dA = emit_dist(cA)
            emit_hist(ab_prev[0])
            kA = emit_kern(dA)
            abA = emit_ab(cA, emit_transp(kA))
            dB = emit_dist(cB)
            emit_hist(ab_prev[1])
            emit_hist(abA)
            kB = emit_kern(dB)
            abB = emit_ab(cB, emit_transp(kB))
            emit_hist(abB)

            hist_sb = opool.tile([128, NV], f32)
            nc.scalar.copy(hist_sb[:], hist[:])
            nc.sync.dma_start(hist_d.ap(), hist_sb[:])

    nc.compile()
    return nc


def _prep_inputs(h_t, cache_h, word_ids):
    h_t = np.asarray(h_t, dtype=np.float32)
    cache_h = np.ascontiguousarray(np.asarray(cache_h, dtype=np.float32))
    word_ids = np.asarray(word_ids, dtype=np.int64)

    # [core, D, RPC] transposed cache slices
    xt8 = np.ascontiguousarray(
        cache_h.reshape(NCORES, RPC, D).transpose(0, 2, 1))

    w8 = word_ids.reshape(NCORES, NCHUNK, GPC, 4, 128)  # [core, ch, g, c, p]
    am8 = np.ascontiguousarray(
        (w8 % 128).astype(np.float32).transpose(0, 4, 1, 3, 2)
    ).reshape(NCORES, 128, BPC)
    bn8 = np.ascontiguousarray(
        (w8 // 128).astype(np.float32).transpose(0, 4, 1, 3, 2)
    ).reshape(NCORES, 128, BPC)

    nh = np.ascontiguousarray(-h_t.reshape(4, 128).T).astype(np.float32)
    im = np.tile(np.arange(128, dtype=np.float16), (128, 1))
    inn = np.tile(np.arange(NV, dtype=np.float16), (128, 1))
    oh = np.zeros((128, 64), np.float16)
    for g in range(8):
        oh[:, g * 8 + g] = 1.0
    idm = np.eye(8, dtype=np.float32)

    in_maps = []
    for k in range(NCORES):
        in_maps.append({
            "xt": xt8[k], "am": am8[k], "bn": bn8[k],
            "nh": nh, "im": im, "inn": inn, "oh": oh, "idm": idm,
        })
    return in_maps


def kernel(h_t, cache_h, word_ids):
    from concourse.bass_utils import run_bass_kernel_spmd

    if "nc" not in _CACHE:
        _CACHE["nc"] = _build_program()
    nc = _CACHE["nc"]

    in_maps = _prep_inputs(h_t, cache_h, word_ids)
    res = run_bass_kernel_spmd(nc, in_maps, list(range(NCORES)))

    hist = np.zeros((128, NV), np.float64)
    for k in range(NCORES):
        hist += res.results[k]["hist"].astype(np.float64)
    cache_p = hist.T.ravel()[:VOCAB]

    m = cache_p.max()
    lse = m + np.log(np.exp(cache_p - m).sum())
    out = (cache_p - lse).astype(np.float32)
    return out[None, :]



# revision 2
# speedup vs baseline: 1.4703x; 1.4703x over previous
"""Trainium2 Bass kernel: cache-distance -> exp kernel -> vocab histogram -> log_softmax.

Math (per cache row i): kern_i = exp(||cache_h[i] - h_t|| / 0.2)
                        cache_p[v] = sum_{i: word_ids[i]==v} kern_i
                        out = log_softmax(cache_p)[None, :]

Device strategy (8 cores, cache sharded along N, DMA-roofline driven):
  - host sorts cache rows by word_id, so the vocab histogram becomes a
    segment-sum over consecutive elements; uploads the cache slice
    pre-transposed [D=512, 32768] in f16 (halves HBM traffic; validated
    numerically: f16-input rel err 2.8e-4 vs the 2e-2 gate)
  - squared distance via one elementwise pass per tile, split between
    two engines to stay under the DMA shadow:
      ACT quarters: (x + (-h))^2 via Square activation w/ per-partition bias
      DVE quarters: z = (x - 2h) * x  (scalar_tensor_tensor, one 2x-mode
        pass), using dist^2 = sum (x-2h)x + ||h||^2; the +||h||^2 lands in
        the Ln activation's free bias
  - PE reduces over D with one-hot-column [128,8] lhsT matmuls into a
    [8, 512] PSUM dist tile per 4096-row chunk
  - ACT: kern = exp(exp(0.5*ln(25*d2 + 25*||h||^2)))  (Ln+Exp+Square share
    one table set; avoids Sqrt)
  - PE transposes kern into [128, 32] batch-major layout, then ONE
    lower-triangular [128,128] matmul per chunk produces within-batch
    (128-element) inclusive prefix sums -- the entire scatter reduced to
    8 tiny matmuls per core
Host: combine per-core prefix tiles (f64 batch-offset cumsum), segment
  diffs at sorted-vocab boundaries, log_softmax (tiny, O(V)).
"""

import os
import sys

for _p in ("/root/.axon_site", "/root/.axon_site/_ro/trn_rl_repo",
           "/root/.axon_site/_ro/pypackages"):
    if os.path.isdir(_p) and _p not in sys.path:
        sys.path.append(_p)

import numpy as np

VOCAB = 50257
N_CACHE = 262144
D = 512
SMOOTH = 0.2
NCORES = 8
RPC = N_CACHE // NCORES        # 32768 rows per core
NCHUNK = 8
CHUNK = RPC // NCHUNK          # 4096 rows per chunk
BPC = RPC // 128               # 256 batches of 128 elements per core

ACT_C = (0,)                   # quarters squared on ACT; rest on DVE

_CACHE = {}


def _patch_act_tables():
    """Restrict the activation table-set chooser to
    natural_log_exp_and_others (covers square/ln/exp/copy) so the whole
    kernel needs exactly one ACT_TABLE_LOAD instead of alternating between
    sets (~2.7us per reload). Set names/order are preserved so
    act_func_set_id indices stay valid."""
    import concourse.hw_specs as hw_specs
    import concourse.bacc as bacc

    if getattr(hw_specs.get_activation_tables, "_histkernel_patched", False):
        return
    orig = hw_specs.get_activation_tables

    def patched(module_arch):
        tabs = orig(module_arch)
        return {
            name: (fns if name == "natural_log_exp_and_others" else set())
            for name, fns in tabs.items()
        }

    patched._histkernel_patched = True
    hw_specs.get_activation_tables = patched
    bacc.get_activation_tables = patched


def _build_program():
    import concourse.bacc as bacc
    import concourse.tile as tile
    import concourse.mybir as mybir

    _patch_act_tables()

    f32, f16 = mybir.dt.float32, mybir.dt.float16
    AF = mybir.ActivationFunctionType
    ALU = mybir.AluOpType

    nc = bacc.Bacc("TRN2", target_bir_lowering=False, debug=False,
                   num_devices=NCORES)

    xt_d = nc.dram_tensor("xt", [D, RPC], f16, kind="ExternalInput")
    nh_d = nc.dram_tensor("nh", [128, 4], f32, kind="ExternalInput")
    h2_d = nc.dram_tensor("h2", [128, 4], f32, kind="ExternalInput")
    lnb_d = nc.dram_tensor("lnb", [8, 1], f32, kind="ExternalInput")
    oh_d = nc.dram_tensor("oh", [128, 64], f16, kind="ExternalInput")
    id_d = nc.dram_tensor("idm", [8, 8], f32, kind="ExternalInput")
    ltr_d = nc.dram_tensor("ltr", [128, 128], f32, kind="ExternalInput")
    pfx_d = nc.dram_tensor("pfx", [128, BPC], f32, kind="ExternalOutput")

    with tile.TileContext(nc) as tc:
        with (
            tc.tile_pool(name="const", bufs=1) as cpool,
            tc.tile_pool(name="x", bufs=8) as xpool,
            tc.tile_pool(name="sq", bufs=6) as sqpool,
            tc.tile_pool(name="s", bufs=6) as spool,
            tc.tile_pool(name="kt", bufs=3) as ktpool,
            tc.tile_pool(name="out", bufs=1) as opool,
            tc.tile_pool(name="psdist", bufs=3, space="PSUM") as psdist,
            tc.tile_pool(name="pskt", bufs=2, space="PSUM") as pskt,
            tc.tile_pool(name="pspfx", bufs=2, space="PSUM") as pspfx,
        ):
            nh = cpool.tile([128, 4], f32)
            nc.sync.dma_start(nh[:], nh_d.ap())
            h2 = cpool.tile([128, 4], f32)
            nc.sync.dma_start(h2[:], h2_d.ap())
            lnb = cpool.tile([8, 1], f32)
            nc.sync.dma_start(lnb[:], lnb_d.ap())
            oh = cpool.tile([128, 64], f16)
            nc.sync.dma_start(oh[:], oh_d.ap())
            idm = cpool.tile([8, 8], f32)
            nc.sync.dma_start(idm[:], id_d.ap())
            ltr = cpool.tile([128, 128], f32)
            nc.sync.dma_start(ltr[:], ltr_d.ap())

            out_sb = opool.tile([128, BPC], f32)
            xt_ap = xt_d.ap()

            def emit_dist(ch):
                dist = psdist.tile([8, 512], f32)
                mm = 0
                for c in range(4):
                    x = xpool.tile([128, CHUNK], f16)
                    nc.sync.dma_start(
                        x[:],
                        xt_ap[c * 128:(c + 1) * 128,
                              ch * CHUNK:(ch + 1) * CHUNK],
                    )
                    z = sqpool.tile([128, CHUNK], f16)
                    if c in ACT_C:
                        nc.scalar.activation(z[:], x[:], AF.Square,
                                             bias=nh[:, c:c + 1])
                    else:
                        nc.vector.scalar_tensor_tensor(
                            z[:], x[:], h2[:, c:c + 1], x[:],
                            ALU.subtract, ALU.mult,
                        )
                    for g in range(8):
                        nc.tensor.matmul(
                            dist[:],
                            oh[:, g * 8:(g + 1) * 8],
                            z[:, g * 512:(g + 1) * 512],
                            start=(mm == 0),
                            stop=(mm == 31),
                        )
                        mm += 1
                return dist

            def emit_post(ch, dist):
                lg = spool.tile([8, 512], f32)
                nc.scalar.activation(lg[:], dist[:], AF.Ln, scale=25.0,
                                     bias=lnb[:, 0:1])
                d5 = spool.tile([8, 512], f32)
                nc.scalar.activation(d5[:], lg[:], AF.Exp, scale=0.5)
                kern = spool.tile([8, 512], f32)
                nc.scalar.activation(kern[:], d5[:], AF.Exp)
                ktp = pskt.tile([128, 32], f32)
                for c4 in range(4):
                    nc.tensor.transpose(
                        ktp[:, c4 * 8:(c4 + 1) * 8],
                        kern[:, c4 * 128:(c4 + 1) * 128],
                        idm[:],
                    )
                kt = ktpool.tile([128, 32], f32)
                nc.vector.tensor_copy(kt[:], ktp[:])
                pf = pspfx.tile([128, 32], f32)
                nc.tensor.matmul(pf[:], ltr[:], kt[:],
                                 start=True, stop=True)
                nc.scalar.copy(out_sb[:, ch * 32:(ch + 1) * 32], pf[:])

            # 1-chunk software stagger: chunk ch's post-dist work (kern
            # chain, transpose, prefix) is emitted behind chunk ch+1's
            # load+dist phase so PE never waits on the serial ACT chain.
            prev = None
            for ch in range(NCHUNK):
                dist = emit_dist(ch)
                if prev is not None:
                    emit_post(prev[0], prev[1])
                prev = (ch, dist)
            emit_post(prev[0], prev[1])

            nc.sync.dma_start(pfx_d.ap(), out_sb[:])

    nc.compile()
    return nc


def _prep_inputs(h_t, cache_h, word_ids):
    h_t = np.asarray(h_t, dtype=np.float32)
    cache_h = np.asarray(cache_h, dtype=np.float32)
    word_ids = np.asarray(word_ids)

    order = np.argsort(word_ids, kind="stable")
    ws = np.asarray(word_ids[order], dtype=np.int64)

    # [core, D, RPC] transposed + sorted cache slices, f16
    xt8 = np.ascontiguousarray(
        cache_h[order].reshape(NCORES, RPC, D).transpose(0, 2, 1)
    ).astype(np.float16)

    hq = h_t.reshape(4, 128).T                      # [128, 4] quarters
    nh = np.ascontiguousarray(-hq).astype(np.float32)
    h2 = np.ascontiguousarray(2.0 * hq).astype(np.float32)
    dve_c = [c for c in range(4) if c not in ACT_C]
    h2d = float(sum((hq[:, c].astype(np.float64) ** 2).sum() for c in dve_c))
    lnb = np.full((8, 1), 25.0 * h2d, np.float32)

    oh = np.zeros((128, 64), np.float16)
    for g in range(8):
        oh[:, g * 8 + g] = 1.0
    idm = np.eye(8, dtype=np.float32)
    ltr = np.triu(np.ones((128, 128), np.float32))  # ltr[p, m] = (p <= m)

    in_maps = []
    for k in range(NCORES):
        in_maps.append({
            "xt": xt8[k], "nh": nh, "h2": h2, "lnb": lnb,
            "oh": oh, "idm": idm, "ltr": ltr,
        })
    return in_maps, ws


def _postprocess(pfx8, ws):
    """pfx8: [8, 128, BPC] within-(128)batch inclusive prefix sums, col
    order (ch, c4, g); ws: sorted word_ids. Returns [1, V] log-softmax."""
    i = np.arange(N_CACHE)
    k = i >> 15
    r = i & 32767
    ch = r >> 12
    rr = r & 4095
    g = rr >> 9
    rrr = rr & 511
    c4 = rrr >> 7
    p = rrr & 127
    col = ch * 32 + c4 * 8 + g

    P_wb = pfx8[k, p, col].astype(np.float64)
    # batch totals in global element order -> exclusive batch offsets
    T = pfx8[k[::128], 127, col[::128]].astype(np.float64)
    off = np.concatenate(([0.0], np.cumsum(T[:-1])))
    G = off[i >> 7] + P_wb          # global inclusive prefix at element i

    counts = np.bincount(ws, minlength=VOCAB)
    ends = np.cumsum(counts) - 1          # inclusive end index per vocab
    starts = ends - counts                # start-1 index per vocab
    Ge = G[np.maximum(ends, 0)]
    Gs = np.where(starts >= 0, G[np.maximum(starts, 0)], 0.0)
    cache_p = np.where(counts > 0, Ge - Gs, 0.0)

    m = cache_p.max()
    lse = m + np.log(np.exp(cache_p - m).sum())
    return (cache_p - lse).astype(np.float32)[None, :]


def kernel(h_t, cache_h, word_ids):
    from concourse.bass_utils import run_bass_kernel_spmd

    if "nc" not in _CACHE:
        _CACHE["nc"] = _build_program()
    nc = _CACHE["nc"]

    in_maps, ws = _prep_inputs(h_t, cache_h, word_ids)
    res = run_bass_kernel_spmd(nc, in_maps, list(range(NCORES)))

    pfx8 = np.stack([res.results[k]["pfx"] for k in range(NCORES)])
    return _postprocess(pfx8, ws)


# revision 11
# speedup vs baseline: 1.8014x; 1.2252x over previous
"""Trainium2 Bass kernel: cache-distance -> exp kernel -> vocab histogram -> log_softmax.

Math (per cache row i): kern_i = exp(||cache_h[i] - h_t|| / 0.2)
                        cache_p[v] = sum_{i: word_ids[i]==v} kern_i
                        out = log_softmax(cache_p)[None, :]

Device strategy (8 cores, cache sharded along N, DMA-roofline driven):
  - host sorts cache rows by word_id, so the vocab histogram becomes a
    segment-sum over consecutive elements; uploads the cache slice
    pre-transposed [D=512, 32768] in f16 (halves HBM traffic; validated
    numerically: f16-input rel err 2.8e-4 vs the 2e-2 gate)
  - squared distance via elementwise squares, split between two engines
    to stay under the DMA shadow (ACT 1x and DVE 4x/2x two-pass rates are
    ~balanced at a 13/19 quarter split):
      ACT quarters: (x + (-h))^2 via Square activation w/ per-partition bias
      DVE quarters: y = x + (-h) (tensor_scalar, 4x mode) then z = y*y
        (tensor_tensor, 2x mode) -- scalar_tensor_tensor would be one pass
        but only has a 1x uop (4480ns measured vs 3321ns for the pair)
  - PE reduces over D with one-hot-column [128,8] lhsT matmuls into a
    [8, 512] PSUM dist tile per 4096-row chunk
  - ACT: kern = exp(exp(0.5*ln(25*d2)))  (Ln+Exp+Square share one table
    set; avoids Sqrt)
  - PE transposes kern into [128, 32] batch-major layout, then ONE
    lower-triangular [128,128] matmul per chunk produces within-batch
    (128-element) inclusive prefix sums -- the entire scatter reduced to
    8 tiny matmuls per core
Host: combine per-core prefix tiles (f64 batch-offset cumsum), segment
  diffs at sorted-vocab boundaries, log_softmax (tiny, O(V)).
"""

import os
import sys

for _p in ("/root/.axon_site", "/root/.axon_site/_ro/trn_rl_repo",
           "/root/.axon_site/_ro/pypackages"):
    if os.path.isdir(_p) and _p not in sys.path:
        sys.path.append(_p)

import numpy as np

VOCAB = 50257
N_CACHE = 262144
D = 512
SMOOTH = 0.2
NCORES = 8
RPC = N_CACHE // NCORES        # 32768 rows per core
NCHUNK = 8
CHUNK = RPC // NCHUNK          # 4096 rows per chunk
BPC = RPC // 128               # 256 batches of 128 elements per core

_CACHE = {}


def _act_owns(ch, c):
    """13 of 32 (ch, c) quarter-tiles squared on ACT, the rest on DVE --
    balances ACT (1x + kern chain) against DVE (4x+2x two-pass)."""
    return c == 0 or (c == 2 and ch % 2 == 0) or (ch == 7 and c == 1)


def _patch_act_tables():
    """Restrict the activation table-set chooser to
    natural_log_exp_and_others (covers square/ln/exp/copy) so the whole
    kernel needs exactly one ACT_TABLE_LOAD instead of alternating between
    sets (~2.7us per reload). Set names/order are preserved so
    act_func_set_id indices stay valid."""
    import concourse.hw_specs as hw_specs
    import concourse.bacc as bacc

    if getattr(hw_specs.get_activation_tables, "_histkernel_patched", False):
        return
    orig = hw_specs.get_activation_tables

    def patched(module_arch):
        tabs = orig(module_arch)
        return {
            name: (fns if name == "natural_log_exp_and_others" else set())
            for name, fns in tabs.items()
        }

    patched._histkernel_patched = True
    hw_specs.get_activation_tables = patched
    bacc.get_activation_tables = patched


def _build_program():
    import concourse.bacc as bacc
    import concourse.tile as tile
    import concourse.mybir as mybir

    _patch_act_tables()

    f32, f16 = mybir.dt.float32, mybir.dt.float16
    AF = mybir.ActivationFunctionType
    ALU = mybir.AluOpType

    nc = bacc.Bacc("TRN2", target_bir_lowering=False, debug=False,
                   num_devices=NCORES)

    xt_d = nc.dram_tensor("xt", [D, RPC], f16, kind="ExternalInput")
    nh_d = nc.dram_tensor("nh", [128, 4], f32, kind="ExternalInput")
    oh_d = nc.dram_tensor("oh", [128, 64], f16, kind="ExternalInput")
    id_d = nc.dram_tensor("idm", [8, 8], f32, kind="ExternalInput")
    ltr_d = nc.dram_tensor("ltr", [128, 128], f32, kind="ExternalInput")
    pfx_d = nc.dram_tensor("pfx", [128, BPC], f32, kind="ExternalOutput")

    with tile.TileContext(nc) as tc:
        with (
            tc.tile_pool(name="const", bufs=1) as cpool,
            tc.tile_pool(name="x", bufs=8) as xpool,
            tc.tile_pool(name="sq", bufs=6) as sqpool,
            tc.tile_pool(name="y", bufs=4) as ypool,
            tc.tile_pool(name="s", bufs=6) as spool,
            tc.tile_pool(name="kt", bufs=3) as ktpool,
            tc.tile_pool(name="out", bufs=1) as opool,
            tc.tile_pool(name="psdist", bufs=3, space="PSUM") as psdist,
            tc.tile_pool(name="pskt", bufs=2, space="PSUM") as pskt,
            tc.tile_pool(name="pspfx", bufs=2, space="PSUM") as pspfx,
        ):
            nh = cpool.tile([128, 4], f32)
            nc.sync.dma_start(nh[:], nh_d.ap())
            oh = cpool.tile([128, 64], f16)
            nc.sync.dma_start(oh[:], oh_d.ap())
            idm = cpool.tile([8, 8], f32)
            nc.sync.dma_start(idm[:], id_d.ap())
            ltr = cpool.tile([128, 128], f32)
            nc.sync.dma_start(ltr[:], ltr_d.ap())

            out_sb = opool.tile([128, BPC], f32)
            xt_ap = xt_d.ap()

            def emit_dist(ch):
                dist = psdist.tile([8, 512], f32)
                mm = 0
                for c in range(4):
                    x = xpool.tile([128, CHUNK], f16)
                    nc.sync.dma_start(
                        x[:],
                        xt_ap[c * 128:(c + 1) * 128,
                              ch * CHUNK:(ch + 1) * CHUNK],
                    )
                    z = sqpool.tile([128, CHUNK], f16)
                    if _act_owns(ch, c):
                        nc.scalar.activation(z[:], x[:], AF.Square,
                                             bias=nh[:, c:c + 1])
                    else:
                        y = ypool.tile([128, CHUNK], f16)
                        nc.vector.tensor_scalar(
                            y[:], x[:], nh[:, c:c + 1], None, ALU.add)
                        nc.vector.tensor_tensor(z[:], y[:], y[:], ALU.mult)
                    for g in range(8):
                        nc.tensor.matmul(
                            dist[:],
                            oh[:, g * 8:(g + 1) * 8],
                            z[:, g * 512:(g + 1) * 512],
                            start=(mm == 0),
                            stop=(mm == 31),
                        )
                        mm += 1
                return dist

            def emit_post(ch, dist):
                lg = spool.tile([8, 512], f32)
                nc.scalar.activation(lg[:], dist[:], AF.Ln, scale=25.0)
                d5 = spool.tile([8, 512], f32)
                nc.scalar.activation(d5[:], lg[:], AF.Exp, scale=0.5)
                kern = spool.tile([8, 512], f32)
                nc.scalar.activation(kern[:], d5[:], AF.Exp)
                ktp = pskt.tile([128, 32], f32)
                for c4 in range(4):
                    nc.tensor.transpose(
                        ktp[:, c4 * 8:(c4 + 1) * 8],
                        kern[:, c4 * 128:(c4 + 1) * 128],
                        idm[:],
                    )
                kt = ktpool.tile([128, 32], f32)
                nc.vector.tensor_copy(kt[:], ktp[:])
                pf = pspfx.tile([128, 32], f32)
                nc.tensor.matmul(pf[:], ltr[:], kt[:],
                                 start=True, stop=True)
                nc.scalar.copy(out_sb[:, ch * 32:(ch + 1) * 32], pf[:])

            # 1-chunk software stagger: chunk ch's post-dist work (kern
            # chain, transpose, prefix) is emitted behind chunk ch+1's
            # load+dist phase so PE never waits on the serial ACT chain.
            prev = None
            for ch in range(NCHUNK):
                dist = emit_dist(ch)
                if prev is not None:
                    emit_post(prev[0], prev[1])
                prev = (ch, dist)
            emit_post(prev[0], prev[1])

            nc.sync.dma_start(pfx_d.ap(), out_sb[:])

    nc.compile()
    return nc


def _prep_inputs(h_t, cache_h, word_ids):
    h_t = np.asarray(h_t, dtype=np.float32)
    cache_h = np.asarray(cache_h, dtype=np.float32)
    word_ids = np.asarray(word_ids)

    order = np.argsort(word_ids, kind="stable")
    ws = np.asarray(word_ids[order], dtype=np.int64)

    # [core, D, RPC] transposed + sorted cache slices, f16
    xt8 = np.ascontiguousarray(
        cache_h[order].reshape(NCORES, RPC, D).transpose(0, 2, 1)
    ).astype(np.float16)

    hq = h_t.reshape(4, 128).T                      # [128, 4] quarters
    nh = np.ascontiguousarray(-hq).astype(np.float32)

    oh = np.zeros((128, 64), np.float16)
    for g in range(8):
        oh[:, g * 8 + g] = 1.0
    idm = np.eye(8, dtype=np.float32)
    ltr = np.triu(np.ones((128, 128), np.float32))  # ltr[p, m] = (p <= m)

    in_maps = []
    for k in range(NCORES):
        in_maps.append({
            "xt": xt8[k], "nh": nh, "oh": oh, "idm": idm, "ltr": ltr,
        })
    return in_maps, ws


def _postprocess(pfx8, ws):
    """pfx8: [8, 128, BPC] within-(128)batch inclusive prefix sums, col
    order (ch, c4, g); ws: sorted word_ids. Returns [1, V] log-softmax."""
    i = np.arange(N_CACHE)
    k = i >> 15
    r = i & 32767
    ch = r >> 12
    rr = r & 4095
    g = rr >> 9
    rrr = rr & 511
    c4 = rrr >> 7
    p = rrr & 127
    col = ch * 32 + c4 * 8 + g

    P_wb = pfx8[k, p, col].astype(np.float64)
    # batch totals in global element order -> exclusive batch offsets
    T = pfx8[k[::128], 127, col[::128]].astype(np.float64)
    off = np.concatenate(([0.0], np.cumsum(T[:-1])))
    G = off[i >> 7] + P_wb          # global inclusive prefix at element i

    counts = np.bincount(ws, minlength=VOCAB)
    ends = np.cumsum(counts) - 1          # inclusive end index per vocab
    starts = ends - counts                # start-1 index per vocab
    Ge = G[np.maximum(ends, 0)]
    Gs = np.where(starts >= 0, G[np.maximum(starts, 0)], 0.0)
    cache_p = np.where(counts > 0, Ge - Gs, 0.0)

    m = cache_p.max()
    lse = m + np.log(np.exp(cache_p - m).sum())
    return (cache_p - lse).astype(np.float32)[None, :]


def kernel(h_t, cache_h, word_ids):
    from concourse.bass_utils import run_bass_kernel_spmd

    if "nc" not in _CACHE:
        _CACHE["nc"] = _build_program()
    nc = _CACHE["nc"]

    in_maps, ws = _prep_inputs(h_t, cache_h, word_ids)
    res = run_bass_kernel_spmd(nc, in_maps, list(range(NCORES)))

    pfx8 = np.stack([res.results[k]["pfx"] for k in range(NCORES)])
    return _postprocess(pfx8, ws)
